# revision 1
# baseline (speedup 1.0000x reference)
"""Causal self-attention (B=4, T=2048, C=1024, H=16, D=64) on 8 trn2 cores.

Sharding: core c = 2*b + g handles batch b and head-group g (8 heads each).
Fully communication-free: each core computes the qkv projection for its head
columns, causal attention for its 8 heads, and a partial output projection
(contraction over its 512 head-columns). The host sums the two head-group
partials per batch and adds out_b.

Device notes (per core):
  - scores are computed TRANSPOSED: sT[k, q]; softmax key-sums ride the PE
    via a ones-augmented V (lhsT = [v | 1]); attention output lands as y^T,
    which feeds the output projection lhsT directly (no transposes anywhere).
  - softmax division is deferred: unnormalized y^T and per-(head,qtile) sum
    rows are staged, one batched reciprocal runs at the end, and K=1 matmuls
    broadcast each reciprocal row across partitions for the final multiply.
  - score PSUM tiles are 2-bank [128, 1024] pairs so exp ops are big.
  - all matmul inputs bf16, accumulation fp32 in PSUM.
"""
import numpy as np
import ml_dtypes
from contextlib import ExitStack

import concourse.bass as bass
import concourse.mybir as mybir
import concourse.tile as tile
from concourse.masks import make_upper_triangular
from concourse.bass_utils import run_bass_kernel_spmd

BF16 = mybir.dt.bfloat16
F32 = mybir.dt.float32

B, T, C = 4, 2048, 1024
H, D = 16, 64
HC = H // 2          # heads per core
P = 128
NQ = 512             # q macro-tile (columns of the transposed score tile)
CK = C // P          # contraction chunks for qkv proj (8)
NT = T // P          # T tiles of 128 (16)
NQT = T // NQ        # q macro tiles (4)
HCOL = HC * D        # head columns per core (512)
NITER = HC * NQT     # attention (head, qtile) iterations (32)


def _split_waits(nc):
    """walrus in this container rejects >1 sync wait per instruction; hoist
    extras onto preceding NoOps on the same engine."""
    for func in nc.m.functions:
        for bb in func.blocks:
            newlist = []
            for inst in bb.instructions:
                si = inst.sync_info
                if si is not None and si.on_wait and len(si.on_wait) > 1:
                    extra = list(si.on_wait[:-1])
                    keep = list(si.on_wait[-1:])
                    for j, w in enumerate(extra):
                        newlist.append(mybir.InstNoOp(
                            name=f"{inst.name}-wsplit{j}",
                            sync_info=mybir.SyncInfo(on_wait=[w], on_update=[]),
                            bass_nofuse=True, engine=inst.engine))
                    si.on_wait = keep
                newlist.append(inst)
            bb.instructions = newlist


def _emit(nc, tc, ctx):
    xT_d = nc.dram_tensor("xT", [C, T], BF16, kind="ExternalInput")
    wqk_d = nc.dram_tensor("wqk", [C, 2 * HCOL], BF16, kind="ExternalInput")
    wv_d = nc.dram_tensor("wv", [C, HCOL], BF16, kind="ExternalInput")
    wout_d = nc.dram_tensor("wout", [HCOL, C], BF16, kind="ExternalInput")
    bqk_d = nc.dram_tensor("bqk", [2 * HCOL], F32, kind="ExternalInput")
    bv_d = nc.dram_tensor("bv", [1, HCOL], F32, kind="ExternalInput")
    out_d = nc.dram_tensor("out", [T, C], F32, kind="ExternalOutput")

    consts = ctx.enter_context(tc.tile_pool(name="consts", bufs=1))
    weights = ctx.enter_context(tc.tile_pool(name="weights", bufs=1))
    acts = ctx.enter_context(tc.tile_pool(name="acts", bufs=1))
    pt_pool = ctx.enter_context(tc.tile_pool(name="ptp", bufs=7))
    misc = ctx.enter_context(tc.tile_pool(name="misc", bufs=4))
    outp = ctx.enter_context(tc.tile_pool(name="outp", bufs=3))
    ps_s = ctx.enter_context(tc.tile_pool(name="ps_s", bufs=3, space="PSUM"))
    ps_av = ctx.enter_context(tc.tile_pool(name="ps_av", bufs=2, space="PSUM"))

    # constants
    tri01 = consts.tile([P, P], BF16, name="tri01")
    make_upper_triangular(nc, tri01, val=1.0, diag=True)
    ones_row = consts.tile([1, P], F32, name="ones_row")
    nc.vector.memset(ones_row, 1.0)
    bqk_sb = consts.tile([P, 2 * HCOL // P], F32, name="bqk_sb")
    nc.sync.dma_start(out=bqk_sb, in_=bqk_d.rearrange("(m p) -> p m", p=P))
    bv_row = consts.tile([1, HCOL], F32, name="bv_row")
    nc.sync.dma_start(out=bv_row, in_=bv_d[:])
    # broadcast v-bias to all 128 partitions via K=1 matmul
    bv_ps = ps_av.tile([P, NQ], F32, name="bv_ps", tag="av")
    nc.tensor.matmul(bv_ps[:, 0:HCOL], lhsT=ones_row, rhs=bv_row,
                     start=True, stop=True)
    bv_full = consts.tile([P, HCOL], F32, name="bv_full")
    nc.vector.tensor_copy(bv_full, bv_ps[:, 0:HCOL])

    # weight/activation loads
    xT_sb = weights.tile([P, CK, T], BF16, name="xT_sb")
    wqk_sb = weights.tile([P, CK, 2 * HCOL], BF16, name="wqk_sb")
    wv_sb = weights.tile([P, CK, HCOL], BF16, name="wv_sb")
    xT_r = xT_d.rearrange("(c p) t -> p c t", p=P)
    wqk_r = wqk_d.rearrange("(c p) n -> p c n", p=P)
    wv_r = wv_d.rearrange("(c p) n -> p c n", p=P)
    for c in range(CK):
        nc.gpsimd.dma_start(out=wqk_sb[:, c], in_=wqk_r[:, c])
        nc.sync.dma_start(out=xT_sb[:, c], in_=xT_r[:, c])
        nc.gpsimd.dma_start(out=wv_sb[:, c], in_=wv_r[:, c])
    wout_sb = weights.tile([P, HCOL // P, C], BF16, name="wout_sb")
    nc.sync.dma_start(out=wout_sb, in_=wout_d.rearrange("(c p) n -> p c n", p=P))

    qkT_sb = acts.tile([P, 2 * HCOL // P, T], BF16, name="qkT_sb")
    v_sb = acts.tile([P, NT, HC, D + 1], BF16, name="v_sb")
    yT_sb = acts.tile([P, HCOL // P, T], BF16, name="yT_sb")
    sums_all = acts.tile([NITER, NQ], F32, name="sums_all")
    recip_all = acts.tile([NITER, NQ], F32, name="recip_all")
    nc.vector.memset(v_sb[:, :, :, D:D + 1], 1.0)

    # ---- QKV projection ----
    # q,k transposed: qkT[col, t]; col-tile m (q: m 0..3, k: m 4..7).
    # Two 512-wide T chunks share one 2-bank psum tile -> one wide bias-act.
    for m in range(2 * HCOL // P):
        for n2 in range(T // (2 * NQ)):
            ps = ps_s.tile([P, 2 * NQ], F32, name="ps_qk", tag="s")
            for half in range(2):
                n = 2 * n2 + half
                for c in range(CK):
                    nc.tensor.matmul(
                        ps[:, half * NQ:(half + 1) * NQ],
                        lhsT=wqk_sb[:, c, m * P:(m + 1) * P],
                        rhs=xT_sb[:, c, n * NQ:(n + 1) * NQ],
                        start=(c == 0), stop=(c == CK - 1))
            nc.scalar.activation(
                qkT_sb[:, m, 2 * n2 * NQ:2 * (n2 + 1) * NQ], ps,
                mybir.ActivationFunctionType.Identity,
                bias=bqk_sb[:, m:m + 1])
    # v natural: v[t, col]; two row-tiles share one psum tile
    for t2 in range(NT // 2):
        ps = ps_s.tile([P, 2 * HCOL], F32, name="ps_v", tag="s")
        for half in range(2):
            t = 2 * t2 + half
            for c in range(CK):
                nc.tensor.matmul(
                    ps[:, half * HCOL:(half + 1) * HCOL],
                    lhsT=xT_sb[:, c, t * P:(t + 1) * P],
                    rhs=wv_sb[:, c, :],
                    start=(c == 0), stop=(c == CK - 1))
        nc.vector.tensor_tensor(
            v_sb[:, 2 * t2:2 * t2 + 2, :, 0:D],
            ps.rearrange("p (tt h d) -> p tt h d", tt=2, h=HC),
            bv_full.rearrange("p (h d) -> p h d", h=HC)[:, None, :, :]
            .to_broadcast((P, 2, HC, D)),
            mybir.AluOpType.add)

    # ---- attention (per head, per q macro-tile) ----
    for h in range(HC):
        po = 64 * (h % 2)
        qT_h = qkT_sb[po:po + D, h // 2, :]
        kT_h = qkT_sb[po:po + D, 4 + h // 2, :]
        for qt in range(NQT):
            it = h * NQT + qt
            diag0 = (qt * NQ) // P      # first diagonal key chunk
            nkc = diag0 + NQ // P       # key chunks needed (causal)
            psum_av = ps_av.tile([P, NQ], F32, name="psum_av", tag="av")
            pts = []   # (pt_tile, half, qoff) per key chunk
            for kc2 in range((nkc + 1) // 2):
                kcs = [kc for kc in (2 * kc2, 2 * kc2 + 1) if kc < nkc]
                ps = ps_s.tile([P, 2 * NQ], F32, name="ps_sc", tag="s")
                pt = pt_pool.tile([P, 2 * NQ], BF16, name="pt", tag="pt")
                qoffs = []
                for half, kc in enumerate(kcs):
                    r = kc - diag0
                    qoff = max(0, r * P)
                    qoffs.append(qoff)
                    nc.tensor.matmul(
                        ps[:, half * NQ + qoff:(half + 1) * NQ],
                        lhsT=kT_h[:, kc * P:(kc + 1) * P],
                        rhs=qT_h[:, qt * NQ + qoff:(qt + 1) * NQ],
                        start=True, stop=True)
                    pts.append((pt, half, qoff))
                if len(kcs) == 2 and qoffs[0] == 0 and qoffs[1] == 0:
                    nc.scalar.activation(
                        pt, ps, mybir.ActivationFunctionType.Exp,
                        scale=float(D) ** -0.5)
                else:
                    for half, kc in enumerate(kcs):
                        qoff = qoffs[half]
                        nc.scalar.activation(
                            pt[:, half * NQ + qoff:(half + 1) * NQ],
                            ps[:, half * NQ + qoff:(half + 1) * NQ],
                            mybir.ActivationFunctionType.Exp,
                            scale=float(D) ** -0.5)
                for half, kc in enumerate(kcs):
                    r = kc - diag0
                    if r >= 0:
                        qoff = qoffs[half]
                        nc.vector.tensor_tensor(
                            pt[:, half * NQ + qoff:half * NQ + qoff + P],
                            pt[:, half * NQ + qoff:half * NQ + qoff + P],
                            tri01, mybir.AluOpType.mult)
                        if qoff > 0:
                            nc.vector.memset(
                                pt[:, half * NQ:half * NQ + qoff], 0.0)
            for kc, (pt, half, _) in enumerate(pts):
                nc.tensor.matmul(
                    psum_av[0:D + 1, :],
                    lhsT=v_sb[:, kc, h, :],
                    rhs=pt[:, half * NQ:(half + 1) * NQ],
                    start=(kc == 0), stop=(kc == nkc - 1))
            # stage unnormalized y^T and the sums row; divide later
            nc.vector.tensor_copy(
                yT_sb[po:po + D, h // 2, qt * NQ:(qt + 1) * NQ],
                psum_av[0:D, :])
            srow = misc.tile([1, NQ], F32, name="srow", tag="srow")
            nc.vector.tensor_copy(srow, psum_av[D:D + 1, :])
            nc.sync.dma_start(out=sums_all[it:it + 1, :], in_=srow)

    # ---- batched reciprocal + deferred normalization ----
    nc.vector.reciprocal(recip_all, sums_all)
    for h in range(HC):
        po = 64 * (h % 2)
        for qt in range(NQT):
            it = h * NQT + qt
            rrow = misc.tile([1, NQ], F32, name="rrow", tag="rrow")
            nc.sync.dma_start(out=rrow, in_=recip_all[it:it + 1, :])
            bps = ps_av.tile([D, NQ], F32, name="bps", tag="av")
            nc.tensor.matmul(bps, lhsT=ones_row[:, 0:D], rhs=rrow,
                             start=True, stop=True)
            ysl = yT_sb[po:po + D, h // 2, qt * NQ:(qt + 1) * NQ]
            nc.vector.tensor_tensor(ysl, ysl, bps, mybir.AluOpType.mult)

    # ---- output projection (partial: this core's 512 head-cols) ----
    for t in range(NT):
        ot = outp.tile([P, C], F32, name="ot", tag="ot")
        ps = ps_s.tile([P, C], F32, name="ps_op", tag="s")
        for half in range(C // NQ):
            for c in range(HCOL // P):
                nc.tensor.matmul(
                    ps[:, half * NQ:(half + 1) * NQ],
                    lhsT=yT_sb[:, c, t * P:(t + 1) * P],
                    rhs=wout_sb[:, c, half * NQ:(half + 1) * NQ],
                    start=(c == 0), stop=(c == HCOL // P - 1))
        nc.scalar.copy(ot, ps)
        nc.sync.dma_start(out=out_d[t * P:(t + 1) * P, :], in_=ot)


_NC = None


def _build():
    global _NC
    if _NC is None:
        nc = bass.Bass("TRN2")
        with tile.TileContext(nc) as tc, ExitStack() as ctx:
            _emit(nc, tc, ctx)
        _split_waits(nc)
        _NC = nc
    return _NC


def _in_maps(x, qkv_w, qkv_b, out_w):
    x = np.asarray(x, np.float32)
    qkv_w = np.asarray(qkv_w, np.float32)
    qkv_b = np.asarray(qkv_b, np.float32)
    out_w = np.asarray(out_w, np.float32)
    maps = []
    xTs = [np.ascontiguousarray(x[b].T).astype(ml_dtypes.bfloat16)
           for b in range(B)]
    for core in range(2 * B):
        b, g = core // 2, core % 2
        lo = g * HCOL
        wq = qkv_w[:, lo:lo + HCOL]
        wk = qkv_w[:, C + lo:C + lo + HCOL]
        wv = qkv_w[:, 2 * C + lo:2 * C + lo + HCOL]
        bq = qkv_b[lo:lo + HCOL]
        bk = qkv_b[C + lo:C + lo + HCOL]
        bv = qkv_b[2 * C + lo:2 * C + lo + HCOL]
        wout = out_w[lo:lo + HCOL, :]
        maps.append({
            "xT": xTs[b],
            "wqk": np.concatenate([wq, wk], 1).astype(ml_dtypes.bfloat16),
            "wv": wv.astype(ml_dtypes.bfloat16),
            "wout": np.ascontiguousarray(wout).astype(ml_dtypes.bfloat16),
            "bqk": np.concatenate([bq, bk]).astype(np.float32),
            "bv": bv[None, :].astype(np.float32),
        })
    return maps


def run(x, qkv_w, qkv_b, out_w, out_b, trace=False, tmpdir=None):
    nc = _build()
    maps = _in_maps(x, qkv_w, qkv_b, out_w)
    res = run_bass_kernel_spmd(nc, maps, core_ids=list(range(2 * B)),
                               trace=trace, tmpdir=tmpdir)
    out_b = np.asarray(out_b, np.float32)
    out = np.empty((B, T, C), np.float32)
    for b in range(B):
        out[b] = res.results[2 * b]["out"] + res.results[2 * b + 1]["out"] \
            + out_b[None, :]
    return out, res


def kernel(x, qkv_w, qkv_b, out_w, out_b):
    out, _ = run(x, qkv_w, qkv_b, out_w, out_b, trace=False)
    return out



# revision 12
# speedup vs baseline: 1.1131x; 1.1131x over previous
"""Causal self-attention (B=4, T=2048, C=1024, H=16, D=64) on 8 trn2 cores.

Sharding: core c = 2*b + g handles batch b and head-group g (8 heads each).
Fully communication-free: each core computes the qkv projection for its head
columns, causal attention for its 8 heads, and a partial output projection
(contraction over its 512 head-columns). The host sums the two head-group
partials per batch and adds out_b.

Device notes (per core):
  - scores are computed TRANSPOSED: sT[k, q]; softmax key-sums ride the PE
    via a ones-augmented V (lhsT = [v | 1]); attention output lands as y^T,
    which feeds the output projection lhsT directly (no transposes anywhere).
  - heads are processed in PAIRS (2j, 2j+1) living at partition offsets 0/64
    of the qkT tile: their K=64 score matmuls land in different PE row
    groups (tile_position auto-derived (0,0)/(64,0)) and execute
    CONCURRENTLY in the 128x128 array.
  - per chunk, one [128, 1024] score psum holds both heads (bank-split);
    one exp instruction covers both (the unwritten mid-gap on diagonal
    chunks is exp'd as garbage but never consumed).
  - A|V matmuls stream only the causally-live q columns (qoff trim), so no
    zero-memsets of pt are needed.
  - softmax normalization is per head-pair: approx reciprocal on the [8,512]
    sums tile, broadcast across partitions with tiny bf16 K=1 matmuls
    (fp32 moving operands stream ~4x slower - keep all matmul inputs bf16).
  - qkv-projection tiles are interleaved into the attention pair loop to
    fill the PE while the scalar engine (exp) is the local bottleneck.
"""
import numpy as np
import ml_dtypes
from contextlib import ExitStack

import concourse.bass as bass
import concourse.mybir as mybir
import concourse.tile as tile
from concourse.masks import make_upper_triangular
from concourse.bass_utils import run_bass_kernel_spmd

BF16 = mybir.dt.bfloat16
F32 = mybir.dt.float32

B, T, C = 4, 2048, 1024
H, D = 16, 64
HC = H // 2          # heads per core (8)
P = 128
NQ = 512             # q macro-tile
CK = C // P          # contraction chunks for qkv proj (8)
NT = T // P          # T tiles of 128 (16)
NQT = T // NQ        # q macro tiles (4)
HCOL = HC * D        # head columns per core (512)
NPAIR = HC // 2      # head pairs per core (4)


def _split_waits(nc):
    """walrus in this container rejects >1 sync wait per instruction; hoist
    extras onto preceding NoOps on the same engine."""
    for func in nc.m.functions:
        for bb in func.blocks:
            newlist = []
            for inst in bb.instructions:
                si = inst.sync_info
                if si is not None and si.on_wait and len(si.on_wait) > 1:
                    extra = list(si.on_wait[:-1])
                    keep = list(si.on_wait[-1:])
                    for j, w in enumerate(extra):
                        newlist.append(mybir.InstNoOp(
                            name=f"{inst.name}-wsplit{j}",
                            sync_info=mybir.SyncInfo(on_wait=[w], on_update=[]),
                            bass_nofuse=True, engine=inst.engine))
                    si.on_wait = keep
                newlist.append(inst)
            bb.instructions = newlist


def _emit(nc, tc, ctx):
    xT_d = nc.dram_tensor("xT", [C, T], BF16, kind="ExternalInput")
    wqk_d = nc.dram_tensor("wqk", [C, 2 * HCOL], BF16, kind="ExternalInput")
    wv_d = nc.dram_tensor("wv", [C, HCOL], BF16, kind="ExternalInput")
    wout_d = nc.dram_tensor("wout", [HCOL, C], BF16, kind="ExternalInput")
    bqk_d = nc.dram_tensor("bqk", [2 * HCOL], F32, kind="ExternalInput")
    bv_d = nc.dram_tensor("bv", [1, HCOL], BF16, kind="ExternalInput")
    out_d = nc.dram_tensor("out", [T, C], F32, kind="ExternalOutput")

    consts = ctx.enter_context(tc.tile_pool(name="consts", bufs=1))
    weights = ctx.enter_context(tc.tile_pool(name="weights", bufs=1))
    acts = ctx.enter_context(tc.tile_pool(name="acts", bufs=1))
    pt_pool = ctx.enter_context(tc.tile_pool(name="ptp", bufs=5))
    misc = ctx.enter_context(tc.tile_pool(name="misc", bufs=4))
    outp = ctx.enter_context(tc.tile_pool(name="outp", bufs=3))
    # PSUM: 8 banks total = ps_sc 2x[128,1024](4) + ps_av 3x[128,512](3)
    #       + ps_aux 1x[128,512](1)
    ps_sc = ctx.enter_context(tc.tile_pool(name="ps_sc", bufs=2, space="PSUM"))
    ps_av = ctx.enter_context(tc.tile_pool(name="ps_av", bufs=3, space="PSUM"))
    ps_aux = ctx.enter_context(tc.tile_pool(name="ps_aux", bufs=1, space="PSUM"))

    # constants
    tri01 = consts.tile([P, P], BF16, name="tri01")
    make_upper_triangular(nc, tri01, val=1.0, diag=True)
    ones_row = consts.tile([1, P], BF16, name="ones_row")
    nc.vector.memset(ones_row, 1.0)
    bqk_sb = consts.tile([P, 2 * HCOL // P], F32, name="bqk_sb")
    nc.sync.dma_start(out=bqk_sb, in_=bqk_d.rearrange("(m p) -> p m", p=P))
    bv_row = consts.tile([1, HCOL], BF16, name="bv_row")
    nc.sync.dma_start(out=bv_row, in_=bv_d[:])
    # broadcast v-bias to all 128 partitions via K=1 matmul (bf16 rhs: fast)
    bv_ps = ps_aux.tile([P, NQ], F32, name="bv_ps", tag="aux")
    nc.tensor.matmul(bv_ps[:, 0:HCOL], lhsT=ones_row, rhs=bv_row,
                     start=True, stop=True)
    bv_full = consts.tile([P, HCOL], F32, name="bv_full")
    nc.vector.tensor_copy(bv_full, bv_ps[:, 0:HCOL])

    # weight/activation loads
    xT_sb = weights.tile([P, CK, T], BF16, name="xT_sb")
    wqk_sb = weights.tile([P, CK, 2 * HCOL], BF16, name="wqk_sb")
    wv_sb = weights.tile([P, CK, HCOL], BF16, name="wv_sb")
    xT_r = xT_d.rearrange("(c p) t -> p c t", p=P)
    wqk_r = wqk_d.rearrange("(c p) n -> p c n", p=P)
    wv_r = wv_d.rearrange("(c p) n -> p c n", p=P)
    for c in range(CK):
        nc.gpsimd.dma_start(out=wqk_sb[:, c], in_=wqk_r[:, c])
        nc.sync.dma_start(out=xT_sb[:, c], in_=xT_r[:, c])
        nc.gpsimd.dma_start(out=wv_sb[:, c], in_=wv_r[:, c])
    wout_sb = weights.tile([P, HCOL // P, C], BF16, name="wout_sb")
    nc.sync.dma_start(out=wout_sb, in_=wout_d.rearrange("(c p) n -> p c n", p=P))

    qkT_sb = acts.tile([P, 2 * HCOL // P, T], BF16, name="qkT_sb")
    v_sb = acts.tile([P, NT, HC, D + 1], BF16, name="v_sb")
    yT_sb = acts.tile([P, NPAIR, T], BF16, name="yT_sb")
    sums_all = acts.tile([2 * HC * NQT // 2, NQ], F32, name="sums_all")
    recip_bf = acts.tile([2 * HC * NQT // 2, NQ], BF16, name="recip_bf")
    nc.vector.memset(v_sb[:, :, :, D:D + 1], 1.0)

    # ---- qkv projection tile emitters (interleaved into attention) ----
    def emit_qk_half(m, n, tag="aux"):
        """qk col-tile m (q: 0-3, k: 4-7), T quarter n: [128, 512] psum."""
        ps = ps_aux.tile([P, NQ], F32, name="ps_qk", tag=tag) if tag == "aux" \
            else ps_av.tile([P, NQ], F32, name="ps_qk", tag=tag)
        for c in range(CK):
            nc.tensor.matmul(
                ps, lhsT=wqk_sb[:, c, m * P:(m + 1) * P],
                rhs=xT_sb[:, c, n * NQ:(n + 1) * NQ],
                start=(c == 0), stop=(c == CK - 1))
        nc.scalar.activation(
            qkT_sb[:, m, n * NQ:(n + 1) * NQ], ps,
            mybir.ActivationFunctionType.Identity, bias=bqk_sb[:, m:m + 1])

    def emit_v_tile(t, tag="aux"):
        """v row-tile t: [128 tokens, 512 head-cols] psum -> v_sb + bias."""
        ps = ps_aux.tile([P, HCOL], F32, name="ps_v", tag=tag) if tag == "aux" \
            else ps_av.tile([P, HCOL], F32, name="ps_v", tag=tag)
        for c in range(CK):
            nc.tensor.matmul(
                ps, lhsT=xT_sb[:, c, t * P:(t + 1) * P], rhs=wv_sb[:, c, :],
                start=(c == 0), stop=(c == CK - 1))
        nc.vector.tensor_tensor(
            v_sb[:, t, :, 0:D],
            ps.rearrange("p (h d) -> p h d", h=HC),
            bv_full.rearrange("p (h d) -> p h d", h=HC),
            mybir.AluOpType.add)

    # background work queue: (deadline_slot, thunk). Slots number the 16
    # (pair, qt) attention blocks; a thunk is emitted before its slot starts.
    # Inside each block one background tile is woven in every 2nd chunk to
    # fill the PE while the scalar engine works through exp.
    bg = []
    for t in range(6, NT):          # v t6..15 needed by (p0, qt t//4)
        bg.append(((t - 4) // 4, lambda t=t: emit_v_tile(t)))
    for j in range(1, NPAIR):       # qk tiles for pair j, spread over pair j-1
        for n in range(4):
            bg.append((4 * (j - 1) + n, lambda j=j, n=n: emit_qk_half(j, n)))
            bg.append((4 * (j - 1) + n,
                       lambda j=j, n=n: emit_qk_half(4 + j, n)))
    bg.sort(key=lambda e: e[0])

    # startup: pair-0 q/k tiles + v t0-5 (alternating psum rings: the av
    # ring is free before attention starts, so startup double-buffers)
    for i, n in enumerate(range(4)):
        emit_qk_half(0, n, tag="aux" if i % 2 else "av")
        emit_qk_half(4, n, tag="av" if i % 2 else "aux")
    for t in range(6):
        emit_v_tile(t, tag="aux" if t % 2 else "av")

    scale = float(D) ** -0.5

    # ---- attention: head pairs (2j, 2j+1) at partition offsets 0/64 ----
    for pair in range(NPAIR):
        hA, hB = 2 * pair, 2 * pair + 1
        for qt in range(NQT):
            slot = 4 * pair + qt
            while bg and bg[0][0] < slot:   # overdue safety net
                bg.pop(0)[1]()
            diag0 = (qt * NQ) // P
            nkc = diag0 + NQ // P
            av_A = ps_av.tile([P, NQ], F32, name="av_A", tag="av")
            av_B = ps_av.tile([P, NQ], F32, name="av_B", tag="av")

            def av_chunk(item, nkc=nkc, av_A=av_A, av_B=av_B, hA=hA, hB=hB):
                pt, kc, qoff = item
                for half, av, h in ((0, av_A, hA), (1, av_B, hB)):
                    nc.tensor.matmul(
                        av[0:D + 1, qoff:NQ],
                        lhsT=v_sb[:, kc, h, :],
                        rhs=pt[:, half * NQ + qoff:(half + 1) * NQ],
                        start=(kc == 0), stop=(kc == nkc - 1))

            pend = []
            for kc in range(nkc):
                if kc % 2 == 1 and bg and bg[0][0] <= slot:
                    bg.pop(0)[1]()
                r = kc - diag0
                qoff = max(0, r * P)
                ps = ps_sc.tile([P, 2 * NQ], F32, name="ps_sc", tag="sc")
                pt = pt_pool.tile([P, 2 * NQ], BF16, name="pt", tag="pt")
                for half, po in ((0, 0), (1, D)):
                    nc.tensor.matmul(
                        ps[:, half * NQ + qoff:(half + 1) * NQ],
                        lhsT=qkT_sb[po:po + D, 4 + pair, kc * P:(kc + 1) * P],
                        rhs=qkT_sb[po:po + D, pair,
                                   qt * NQ + qoff:(qt + 1) * NQ],
                        start=True, stop=True)
                # one exp spans both heads; the unwritten [NQ:NQ+qoff] gap
                # holds garbage that no A|V matmul ever reads.
                nc.scalar.activation(
                    pt[:, qoff:2 * NQ], ps[:, qoff:2 * NQ],
                    mybir.ActivationFunctionType.Exp, scale=scale)
                if r >= 0:
                    for half in range(2):
                        o = half * NQ + qoff
                        nc.vector.tensor_tensor(
                            pt[:, o:o + P], pt[:, o:o + P], tri01,
                            mybir.AluOpType.mult)
                # pipeline A|V one chunk behind so exp(kc) hides under the
                # score matmuls of kc+1.
                pend.append((pt, kc, qoff))
                if len(pend) > 1:
                    av_chunk(pend.pop(0))
            for item in pend:
                av_chunk(item)
            # stage unnormalized y^T and the key-sum rows; division deferred
            nc.vector.tensor_copy(
                yT_sb[0:D, pair, qt * NQ:(qt + 1) * NQ], av_A[0:D, :])
            nc.vector.tensor_copy(
                yT_sb[D:2 * D, pair, qt * NQ:(qt + 1) * NQ], av_B[0:D, :])
            for half, av in ((0, av_A), (1, av_B)):
                srow = misc.tile([1, NQ], F32, name="srow", tag="srow")
                nc.vector.tensor_copy(srow, av[D:D + 1, :])
                it = qt * 2 * NPAIR + pair * 2 + half
                nc.sync.dma_start(out=sums_all[it:it + 1, :], in_=srow)

    # ---- deferred softmax division + output projection, interleaved ----
    # one batched reciprocal, cast to bf16; per qt quarter: broadcast the
    # recip rows across partitions with tiny bf16 K=1 matmuls (col-tiled
    # halves run concurrently), normalize each pair's yT slice in place,
    # then immediately emit the quarter's 4 output-projection tiles.
    recip_all = misc.tile([2 * HC * NQT // 2, NQ], F32, name="recip_all",
                          tag="recip", bufs=1)
    nc.vector.reciprocal(recip_all, sums_all)
    nc.vector.tensor_copy(recip_bf, recip_all)
    for qt in range(NQT):
        for pair in range(NPAIR):
            rbA = misc.tile([1, NQ], BF16, name="rbA", tag="rb")
            rbB = misc.tile([1, NQ], BF16, name="rbB", tag="rb")
            it = qt * 2 * NPAIR + pair * 2
            nc.sync.dma_start(out=rbA, in_=recip_bf[it:it + 1, :])
            nc.sync.dma_start(out=rbB, in_=recip_bf[it + 1:it + 2, :])
            bps = ps_av.tile([P, NQ], F32, name="bps", tag="av")
            nc.tensor.matmul(bps[0:D, :], lhsT=ones_row[:, 0:D], rhs=rbA,
                             start=True, stop=True)
            nc.tensor.matmul(bps[D:P, :], lhsT=ones_row[:, 0:D], rhs=rbB,
                             start=True, stop=True)
            ysl = yT_sb[:, pair, qt * NQ:(qt + 1) * NQ]
            nc.vector.tensor_tensor(ysl, ysl, bps, mybir.AluOpType.mult)
        for t in range(4 * qt, 4 * qt + 4):
            ot = outp.tile([P, C], F32, name="ot", tag="ot")
            ps = ps_sc.tile([P, C], F32, name="ps_op", tag="sc")
            for half in range(C // NQ):
                for c in range(HCOL // P):
                    nc.tensor.matmul(
                        ps[:, half * NQ:(half + 1) * NQ],
                        lhsT=yT_sb[:, c, t * P:(t + 1) * P],
                        rhs=wout_sb[:, c, half * NQ:(half + 1) * NQ],
                        start=(c == 0), stop=(c == HCOL // P - 1))
            if t % 2 == 0:
                nc.vector.tensor_copy(ot, ps)
            else:
                nc.scalar.copy(ot, ps)
            nc.sync.dma_start(out=out_d[t * P:(t + 1) * P, :], in_=ot)


_NC = None


def _build():
    global _NC
    if _NC is None:
        nc = bass.Bass("TRN2")
        with tile.TileContext(nc) as tc, ExitStack() as ctx:
            _emit(nc, tc, ctx)
        _split_waits(nc)
        _NC = nc
    return _NC


def _in_maps(x, qkv_w, qkv_b, out_w):
    x = np.asarray(x, np.float32)
    qkv_w = np.asarray(qkv_w, np.float32)
    qkv_b = np.asarray(qkv_b, np.float32)
    out_w = np.asarray(out_w, np.float32)
    maps = []
    xTs = [np.ascontiguousarray(x[b].T).astype(ml_dtypes.bfloat16)
           for b in range(B)]
    for core in range(2 * B):
        b, g = core // 2, core % 2
        lo = g * HCOL
        wq = qkv_w[:, lo:lo + HCOL]
        wk = qkv_w[:, C + lo:C + lo + HCOL]
        wv = qkv_w[:, 2 * C + lo:2 * C + lo + HCOL]
        bq = qkv_b[lo:lo + HCOL]
        bk = qkv_b[C + lo:C + lo + HCOL]
        bv = qkv_b[2 * C + lo:2 * C + lo + HCOL]
        wout = out_w[lo:lo + HCOL, :]
        maps.append({
            "xT": xTs[b],
            "wqk": np.concatenate([wq, wk], 1).astype(ml_dtypes.bfloat16),
            "wv": wv.astype(ml_dtypes.bfloat16),
            "wout": np.ascontiguousarray(wout).astype(ml_dtypes.bfloat16),
            "bqk": np.concatenate([bq, bk]).astype(np.float32),
            "bv": bv[None, :].astype(ml_dtypes.bfloat16),
        })
    return maps


def run(x, qkv_w, qkv_b, out_w, out_b, trace=False, tmpdir=None):
    nc = _build()
    maps = _in_maps(x, qkv_w, qkv_b, out_w)
    res = run_bass_kernel_spmd(nc, maps, core_ids=list(range(2 * B)),
                               trace=trace, tmpdir=tmpdir)
    out_b = np.asarray(out_b, np.float32)
    out = np.empty((B, T, C), np.float32)
    for b in range(B):
        out[b] = res.results[2 * b]["out"] + res.results[2 * b + 1]["out"] \
            + out_b[None, :]
    return out, res


def kernel(x, qkv_w, qkv_b, out_w, out_b):
    out, _ = run(x, qkv_w, qkv_b, out_w, out_b, trace=False)
    return out
